# revision 13
# baseline (speedup 1.0000x reference)
"""Trainium2 Bass kernel for nn_DropoutBlock (gnn_message_passing).

Computes two rounds of sparse-conv (gather + GEMM) + eval-mode BatchNorm + ReLU:
    h   = relu(bn1(sum_k x[nbr[:, k]] @ W1[k]))
    out = relu(bn2(sum_k h[nbr[:, k]] @ W2[k]))

Sharding: data-parallel over rows across 8 NeuronCores.  x (64 MB) is
replicated; each core computes a 125k-row shard of h, an on-device AllGather
replicates h, then each core computes its shard of the output.

Measured 2026-08-08: full-scale relative error 3.6e-07 vs the jax
reference; 47.4 ms device time per pass with the 52,920 indirect gathers
round-robined over all 4 SWDGE queues (num_swdge_queues=4,
queue=qPoolDynamic{0-3}); 67.3 ms on one queue.

Why 128 gathered rows per instruction is a hard ceiling (and why the
obvious "batch more offsets per instruction" fails): the DGE indirect1d
command encodes src/dst as DMA_1D_TENSORs, so the ucode's lockstep walk
pairs exactly one index per dest partition. A [128, M] offset AP is NOT
consumed as 128*M gather indices - the ucode takes column 0 as the per-
partition index and reads out.size/128 CONTIGUOUS elements from each
indexed row (verified empirically: dest chunk m receives row idx[p,0]+m).
Multi-queue works (4 Q7 pairs generate descriptors in parallel) but gains
only ~1.4x because every indirect instruction's index fetch does a
barrier+allgather across all 8 Q7 cores, a shared serial ~0.9 us floor.

Device implementation per 512-row group:
  - 108 indirect DMA gathers (128 rows x 64/128 B each) from x / h_full,
    round-robined across the 4 SWDGE queues
  - PE transposes of the gathered [row, feature] tiles into [feature, row]
  - matmuls against chunk-stacked, BN-scale-folded weights -> PSUM [32, 512]
    (chunk widths 128/128/128/48 for layer 1, 128x6/96 for layer 2 - the
    gather destination is unpadded, so no pad memset is needed)
  - fused bias+ReLU on the scalar engine (BN shift as per-partition bias)
  - PE back-transposes -> [row, 32] -> one 512 B/partition DMA store
"""
import sys

sys.path.insert(0, "/opt/trn_rl_repo")

import contextlib

import numpy as np

import concourse.bass as bass
import concourse.bacc as bacc
import concourse.tile as tile
from concourse import mybir
from concourse.bass_utils import run_bass_kernel_spmd
from concourse.masks import make_identity

P = 128
N = 1_000_000
CIN, COUT, K = 16, 32, 27
NCORES = 8
NSH = N // NCORES  # 125000
T = 4  # row-tiles per group; row r = base + p*T + t
ROWS = P * T  # 512 rows per group
NGRP_FULL = -(-NSH // ROWS)  # 245
NSH_PAD = NGRP_FULL * ROWS  # 125440
NFULL_PAD = NCORES * NSH_PAD

KC1 = K * CIN  # 432; chunk widths 128,128,128,48
NCH1 = 4
KC2 = K * COUT  # 864; chunk widths 128x6,96
NCH2 = 7

F32 = mybir.dt.float32
I32 = mybir.dt.int32
EPS = 1e-5


def _chunk_pack(wcat, nchunk):
    """[nchunk*128, 32] -> [128, nchunk, 32] with chunk j at [:, j, :]."""
    return np.ascontiguousarray(
        wcat.reshape(nchunk, P, COUT).transpose(1, 0, 2)
    )


UNROLL = 5  # groups per hardware-loop iteration (245 = 49 * 5)


NSWQ = 4  # SWDGE queues; indirect gathers round-robin across qPoolDynamic{0-3}


def _build_nc(ngrp, rep=1):
    nc = bacc.Bacc("TRN2", target_bir_lowering=False, debug=False,
                   num_devices=NCORES, num_swdge_queues=NSWQ)
    x_d = nc.dram_tensor("x", [N, CIN], F32, kind="ExternalInput")
    nbr1_d = nc.dram_tensor("nbr1", [NSH_PAD, K], I32, kind="ExternalInput")
    nbr2_d = nc.dram_tensor("nbr2", [NSH_PAD, K], I32, kind="ExternalInput")
    w1_d = nc.dram_tensor("wcat1", [P, NCH1, COUT], F32, kind="ExternalInput")
    w2_d = nc.dram_tensor("wcat2", [P, NCH2, COUT], F32, kind="ExternalInput")
    sh1_d = nc.dram_tensor("shift1", [COUT, 1], F32, kind="ExternalInput")
    sh2_d = nc.dram_tensor("shift2", [COUT, 1], F32, kind="ExternalInput")
    out_d = nc.dram_tensor("out", [NSH_PAD, COUT], F32, kind="ExternalOutput")
    hsh_d = nc.dram_tensor("hsh", [NSH_PAD, COUT], F32)
    hfull_d = nc.dram_tensor("hfull", [NFULL_PAD, COUT], F32,
                             addr_space="Shared")

    with tile.TileContext(nc) as tc, contextlib.ExitStack() as ctx:
        const = ctx.enter_context(tc.tile_pool(name="const", bufs=1))
        sbuf = ctx.enter_context(tc.tile_pool(name="sbuf", bufs=3))
        gpool = ctx.enter_context(tc.tile_pool(name="gpool", bufs=3))
        psum = ctx.enter_context(tc.tile_pool(name="psum", bufs=2, space="PSUM"))
        psacc = ctx.enter_context(tc.tile_pool(name="psacc", bufs=2,
                                               space="PSUM"))

        ident = const.tile([P, P], F32)
        make_identity(nc, ident[:])
        w1_sb = const.tile([P, NCH1 * COUT], F32)
        nc.sync.dma_start(out=w1_sb[:].rearrange("p (j c) -> p j c", c=COUT),
                          in_=w1_d[:])
        w2_sb = const.tile([P, NCH2 * COUT], F32)
        nc.sync.dma_start(out=w2_sb[:].rearrange("p (j c) -> p j c", c=COUT),
                          in_=w2_d[:])
        sh1_sb = const.tile([COUT, 1], F32)
        nc.sync.dma_start(out=sh1_sb[:], in_=sh1_d[:])
        sh2_sb = const.tile([COUT, 1], F32)
        nc.sync.dma_start(out=sh2_sb[:], in_=sh2_d[:])

        def group_body(base, nbr_d, src_d, cin, kc, nchunk, w_sb,
                       sh_sb, dst_d):
            nbr_sb = sbuf.tile([P, T * K], I32, tag="nbr")
            nc.sync.dma_start(
                out=nbr_sb[:].rearrange("p (t w) -> p t w", w=K),
                in_=nbr_d[bass.ds(base, ROWS), :].rearrange(
                    "(p t) w -> p t w", t=T),
            )
            g = gpool.tile([P, T * kc], F32, tag=f"g{kc}")
            for t in range(T):
                for k in range(K):
                    inst = nc.gpsimd.indirect_dma_start(
                        out=g[:, t * kc + k * cin:t * kc + (k + 1) * cin],
                        out_offset=None,
                        in_=src_d[:],
                        in_offset=bass.IndirectOffsetOnAxis(
                            ap=nbr_sb[:, t * K + k:t * K + k + 1], axis=0),
                    )
                    qn = t % NSWQ  # 27-instruction runs per queue
                    if qn:
                        inst.ins.queue = f"qPoolDynamic{qn}"
            ot = psacc.tile([COUT, ROWS], F32, tag="ot")
            for j in range(nchunk):
                w0 = j * P
                cw = min((j + 1) * P, kc) - w0
                tch = sbuf.tile([P, ROWS], F32, tag="tch")
                for t in range(T):
                    pt = psum.tile([P, P], F32, tag="pt")
                    nc.tensor.transpose(
                        out=pt[:cw, :],
                        in_=g[:, t * kc + w0:t * kc + w0 + cw],
                        identity=ident[:],
                    )
                    nc.vector.tensor_copy(out=tch[:cw, t * P:(t + 1) * P],
                                          in_=pt[:cw, :])
                nc.tensor.matmul(
                    out=ot[:],
                    lhsT=w_sb[:cw, j * COUT:(j + 1) * COUT],
                    rhs=tch[:cw, :],
                    start=(j == 0),
                    stop=(j == nchunk - 1),
                )
            ht = sbuf.tile([COUT, ROWS], F32, tag="ht")
            nc.scalar.activation(out=ht[:], in_=ot[:],
                                 func=mybir.ActivationFunctionType.Relu,
                                 bias=sh_sb[:])
            osb = sbuf.tile([P, T * COUT], F32, tag="osb")
            for t in range(T):
                bt = psum.tile([P, COUT], F32, tag="bt")
                nc.tensor.transpose(out=bt[:], in_=ht[:, t * P:(t + 1) * P],
                                    identity=ident[:COUT, :COUT])
                nc.vector.tensor_copy(out=osb[:, t * COUT:(t + 1) * COUT],
                                      in_=bt[:])
            nc.sync.dma_start(
                out=dst_d[bass.ds(base, ROWS), :].rearrange(
                    "(p t) c -> p t c", t=T),
                in_=osb[:].rearrange("p (t c) -> p t c", c=COUT),
            )

        def layer(nbr_d, src_d, cin, kc, nchunk, w_sb, sh_sb, dst_d):
            unroll = min(UNROLL, ngrp)
            assert ngrp % unroll == 0
            niter = ngrp // unroll
            rep_ctx = tc.For_i(0, rep) if rep > 1 else contextlib.nullcontext()
            with rep_ctx:
                with tc.For_i(0, niter,
                              hint_engines=(mybir.EngineType.Pool,)) as i:
                    for u in range(unroll):
                        base = i * (unroll * ROWS) + u * ROWS
                        group_body(base, nbr_d, src_d, cin, kc,
                                   nchunk, w_sb, sh_sb, dst_d)

        layer(nbr1_d, x_d, CIN, KC1, NCH1, w1_sb, sh1_sb, hsh_d)

        nc.gpsimd.collective_compute(
            "AllGather",
            mybir.AluOpType.bypass,
            replica_groups=[list(range(NCORES))],
            ins=[hsh_d[:]],
            outs=[hfull_d[:]],
        )

        layer(nbr2_d, hfull_d, COUT, KC2, NCH2, w2_sb, sh2_sb, out_d)

    nc.compile()
    return nc


_NC_CACHE = {}


def _get_nc(ngrp, rep=1):
    if (ngrp, rep) not in _NC_CACHE:
        _NC_CACHE[(ngrp, rep)] = _build_nc(ngrp, rep)
    return _NC_CACHE[(ngrp, rep)]


def _prep_inputs(x, nbr, W1, gamma1, beta1, mean1, var1, W2, gamma2, beta2,
                 mean2, var2):
    x = np.asarray(x, np.float32)
    nbr = np.asarray(nbr, np.int32)
    scale1 = np.asarray(gamma1) / np.sqrt(np.asarray(var1) + EPS)
    shift1 = np.asarray(beta1) - np.asarray(mean1) * scale1
    scale2 = np.asarray(gamma2) / np.sqrt(np.asarray(var2) + EPS)
    shift2 = np.asarray(beta2) - np.asarray(mean2) * scale2

    w1cat = np.zeros((NCH1 * P, COUT), np.float32)
    w1cat[:KC1] = np.asarray(W1, np.float32).reshape(KC1, COUT) * scale1[None]
    w2cat = np.zeros((NCH2 * P, COUT), np.float32)
    w2cat[:KC2] = np.asarray(W2, np.float32).reshape(KC2, COUT) * scale2[None]
    w1p = _chunk_pack(w1cat, NCH1)
    w2p = _chunk_pack(w2cat, NCH2)
    sh1 = np.ascontiguousarray(shift1.astype(np.float32)[:, None])
    sh2 = np.ascontiguousarray(shift2.astype(np.float32)[:, None])

    # L2 indices: position of global row j inside the padded AllGather layout
    nbr2_full = (nbr // NSH) * NSH_PAD + nbr % NSH

    in_maps = []
    for c in range(NCORES):
        sl = slice(c * NSH, (c + 1) * NSH)
        pad = ((0, NSH_PAD - NSH), (0, 0))
        in_maps.append({
            "x": x,
            "nbr1": np.ascontiguousarray(np.pad(nbr[sl], pad)),
            "nbr2": np.ascontiguousarray(np.pad(nbr2_full[sl], pad)),
            "wcat1": w1p,
            "wcat2": w2p,
            "shift1": sh1,
            "shift2": sh2,
        })
    return in_maps


def run(trace=False, ngrp=NGRP_FULL, rep=1, **inputs):
    nc = _get_nc(ngrp, rep)
    in_maps = _prep_inputs(**inputs)
    # Devices occasionally wedge transiently (NRT_EXEC_UNIT_UNRECOVERABLE on a
    # known-good kernel); one retry recovers in practice.
    try:
        res = run_bass_kernel_spmd(nc, in_maps, core_ids=list(range(NCORES)),
                                   trace=trace)
    except Exception:
        import time as _time
        _time.sleep(5.0)
        res = run_bass_kernel_spmd(nc, in_maps, core_ids=list(range(NCORES)),
                                   trace=trace)
    out = np.concatenate(
        [res.results[c]["out"][:NSH] for c in range(NCORES)], axis=0)
    return out, res


def kernel(**inputs):
    out, _ = run(trace=False, **inputs)
    return out

